# revision 38
# baseline (speedup 1.0000x reference)
"""Single-head causal attention (B=4, T=4096, C=768, H=64) on 8 NeuronCores.

Sharding: 2 cores per batch. Within a batch the 4096 keys are split between
the two cores by interleaved 128-row blocks (core parity p takes global key
blocks {2g+p}).  Every core computes partial attention (un-normalized
numerator + denominator) for ALL 4096 queries of its batch over ITS 2048
keys; the host adds the two partials and normalizes.  This makes the causal
work exactly equal on all 8 cores and the device program identical (all
core-dependence lives in the input data, including the diagonal masks).

Device program (per core), all matmul operands bf16, accumulation fp32:
  qT  [64, 512]  = Wq^T @ x^T per q-tile j     (x^T supplied as [128,6,T])
  kvT [128, 512] = [Wk|Wv]^T @ x_own^T per own 512-key block
  v   [s,64]     = PE-transpose of vT rows, + ones column -> v' [s,65]
  per q-tile j (512 queries), own key chunks gg <= 2j+1 (128 keys each):
     scoresT[s,q] = kT_g^T @ qT_j  (PSUM [128, 512] per chunk)
     non-diagonal groups of 2 chunks: exp(s/8) alternating between
       ScalarE (real exp) and DVE (one-op Schraudolph exp straight to
       bf16: i16 = s*A/2^16 + B/2^16; the int16 bit pattern IS the bf16
       value).  The diagonal group (chunks 2j, 2j+1) uses ScalarE exp +
       a DVE multiply by the causal mask; chunk 2j+1 only needs
       q-subblocks d>=2 (256 cols).
  out flip: ops[128q, 65] += w[128s, 128q-slice]^T @ v'[128s, 65]
     accumulated over chunks per q-subblock d (cols 65d..65d+64; col 64 =
     denominator via the ones column).  One PSUM group per j-tile (start on
     first emitted matmul, stop on last; zero-region = the whole 2KB bank).

Scheduling: the out-matmuls of each score-group are deferred two groups so
the static PE stream never waits inline for an exp; the last j-tile runs
its diagonal group first to shorten the tail; dummy warm-up matmuls run
during the initial DMA wait so the PE reaches full clock before real work;
inputs stream in as a few large, arrival-ordered DMAs.
"""

import sys

for _p in ("/opt/trn_rl_repo",):
    if _p not in sys.path:
        sys.path.insert(0, _p)

import math
import numpy as np
import ml_dtypes

import concourse.bass as bass
import concourse.mybir as mybir
import concourse.tile as tile
from concourse import bacc
from concourse import bass_utils
from concourse.masks import make_identity

BF16 = mybir.dt.bfloat16
F32 = mybir.dt.float32
I32 = mybir.dt.int32
I16 = mybir.dt.int16

P = 128
T = 4096
C = 768
H = 64
CC = C // P        # 6 contraction chunks
OWN = T // 2       # own keys per core
NJ = T // 512      # 8 q-tiles
NCORES = 8

# Schraudolph exp(s/8) straight to bf16: i16 = s*SA16 + SB16 truncated;
# the int16 bit pattern is the bf16 value (top half of the f32 word).
SA16 = float((1 << 23) / (8.0 * math.log(2.0)) / 65536.0)
SB16 = float((127 * (1 << 23) - 366000) / 65536.0 + 0.5)

_NC_CACHE = {}


def _build_nc():
    nc = bacc.Bacc("TRN2", target_bir_lowering=False, debug=False,
                   num_devices=NCORES)

    x6 = nc.dram_tensor("x6", [P, CC, T], BF16, kind="ExternalInput")
    xo6 = nc.dram_tensor("xo6", [P, CC, OWN], BF16, kind="ExternalInput")
    wq = nc.dram_tensor("wq", [P, CC * 64], BF16, kind="ExternalInput")
    wkv = nc.dram_tensor("wkv", [P, CC * 128], BF16, kind="ExternalInput")
    mskb = nc.dram_tensor("mskb", [P, 768], BF16, kind="ExternalInput")
    outp = nc.dram_tensor("outp", [NJ, P, 4 * 65], F32, kind="ExternalOutput")

    with tile.TileContext(nc) as tc:
        with (
            tc.tile_pool(name="const", bufs=1) as cst,
            tc.tile_pool(name="big", bufs=1) as big,
            tc.tile_pool(name="pps", bufs=2, space="PSUM") as pps,
            tc.tile_pool(name="sps", bufs=2, space="PSUM") as sps_pool,
            tc.tile_pool(name="ops", bufs=2, space="PSUM") as ops_pool,
            tc.tile_pool(name="wt", bufs=4) as wt_pool,
            tc.tile_pool(name="wtd", bufs=2) as wtd_pool,
            tc.tile_pool(name="si", bufs=2) as si_pool,
            tc.tile_pool(name="osb", bufs=4) as osb_pool,
        ):
            ident = cst.tile([P, P], BF16)
            make_identity(nc, ident[:])
            wkv_sb = cst.tile([P, CC * 128], BF16)
            nc.sync.dma_start(wkv_sb[:], wkv[:])
            wq_sb = cst.tile([P, CC * 64], BF16)
            mskb_sb = cst.tile([P, 768], BF16)

            xt = big.tile([P, CC, T], BF16, tag="xt")
            xto = big.tile([P, CC, OWN], BF16, tag="xto")

            # Paced input stream (q-tile j: xt cols [512j,512j+512); kv
            # block tb: xto cols [512tb,512tb+512)).  wkv/xto_b0 lead so the
            # kv-projection can start first; wq/msk follow xt_j0.
            nc.sync.dma_start(xto[:, :, 0:512], xo6[:, :, 0:512])
            nc.sync.dma_start(xt[:, :, 0:512], x6[:, :, 0:512])
            nc.sync.dma_start(wq_sb[:], wq[:])
            nc.sync.dma_start(xt[:, :, 512:1024], x6[:, :, 512:1024])
            nc.sync.dma_start(mskb_sb[:], mskb[:])
            for dst, src, a, b in (
                (xt, x6, 1024, 1536),
                (xto, xo6, 512, 1024),
                (xt, x6, 1536, 2048),
                (xt, x6, 2048, 2560),
                (xto, xo6, 1024, 1536),
                (xt, x6, 2560, 3072),
                (xt, x6, 3072, 3584),
                (xto, xo6, 1536, 2048),
                (xt, x6, 3584, 4096),
            ):
                nc.sync.dma_start(dst[:, :, a:b], src[:, :, a:b])

            # PE warm-up during the initial DMA wait: cheap matmuls on the
            # identity keep the PE continuously busy so it ramps to full
            # clock before real data arrives.  Results are discarded.
            wps = pps.tile([P, 512], F32, tag="pps")
            for i in range(30):
                nc.tensor.matmul(wps[:, 0:P], ident[:], ident[:],
                                 start=True, stop=True)

            kvts = []
            vsbs = []

            def emit_kv_block(tb):
                ps = pps.tile([P, 512], F32, tag="pps")
                for ci in range(CC):
                    nc.tensor.matmul(
                        ps[:], wkv_sb[:, 128 * ci:128 * (ci + 1)],
                        xto[:, ci, 512 * tb:512 * (tb + 1)],
                        start=(ci == 0), stop=(ci == CC - 1))
                kvt = big.tile([P, 512], BF16, tag=f"kvT{tb}")
                nc.vector.tensor_copy(kvt[:], ps[:])
                kvts.append(kvt)
                # v' tiles for the 4 chunks of this block
                vsb = big.tile([P, 4 * 65], BF16, tag=f"v{tb}")
                nc.gpsimd.memset(vsb[:], 1.0)
                vp = pps.tile([P, 512], BF16, tag="pps")
                for i in range(4):
                    nc.tensor.transpose(
                        vp[:, 64 * i:64 * (i + 1)],
                        kvt[64:128, 128 * i:128 * (i + 1)],
                        ident[64:128, 64:128])
                for i in range(4):
                    nc.vector.tensor_copy(vsb[:, 65 * i:65 * i + 64],
                                          vp[:, 64 * i:64 * (i + 1)])
                vsbs.append(vsb)

            def warm_fill(n):
                w = pps.tile([P, 512], F32, tag="pps")
                for _ in range(n):
                    nc.tensor.matmul(w[:, 0:P], ident[:], ident[:],
                                     start=True, stop=True)

            # Deferred out-matmul work items, flushed two score-groups late
            # so the PE stream never waits inline on an exp.
            pending = []

            def flush_pending(limit):
                while len(pending) > limit:
                    pending.pop(0)()

            def out_mm(st, ops, j, gg, d, lhsT, stop):
                nc.tensor.matmul(
                    ops[:, 65 * d:65 * (d + 1)], lhsT,
                    vsbs[gg // 4][:, 65 * (gg % 4):65 * (gg % 4 + 1)],
                    start=not st["started"], stop=stop)
                st["started"] = True

            for j in range(NJ):
                st = {"started": False}

                def finish_j(ops, j):
                    osb = osb_pool.tile([P, 4 * 65], F32, tag="osb")
                    if j >= 6:
                        nc.vector.tensor_copy(osb[:], ops[:])
                    else:
                        nc.scalar.copy(osb[:], ops[:])
                    nc.sync.dma_start(outp[j], osb[:])

                def emit_scores_nondiag(qt, g, j, dve=False):
                    sp = sps_pool.tile([P, 1024], F32, tag="sps")
                    for i in range(2):
                        gg = 2 * g + i
                        nc.tensor.matmul(
                            sp[:, 512 * i:512 * (i + 1)],
                            kvts[gg // 4][0:64, 128 * (gg % 4):128 * (gg % 4 + 1)],
                            qt[:], start=True, stop=True)
                    wt = wt_pool.tile([P, 1024], BF16, tag="wt")
                    if dve:
                        nc.vector.tensor_scalar(
                            wt[:].bitcast(I16), sp[:], SA16, SB16,
                            mybir.AluOpType.mult, mybir.AluOpType.add)
                    else:
                        nc.scalar.activation(
                            wt[:], sp[:], mybir.ActivationFunctionType.Exp,
                            scale=1.0 / math.sqrt(H))
                    return wt

                def outs_nondiag(ops, j, g, wt, last, st=st):
                    def fn():
                        for i in range(2):
                            gg = 2 * g + i
                            for d in range(4):
                                out_mm(st, ops, j, gg, d,
                                       wt[:, 512 * i + 128 * d:
                                          512 * i + 128 * (d + 1)],
                                       stop=(last and i == 1 and d == 3))
                        if last:
                            finish_j(ops, j)
                    return fn

                def emit_diag(qt, j):
                    sp = sps_pool.tile([P, 1024], F32, tag="sps")
                    gg0, gg1 = 2 * j, 2 * j + 1
                    nc.tensor.matmul(
                        sp[:, 0:512],
                        kvts[gg0 // 4][0:64, 128 * (gg0 % 4):128 * (gg0 % 4 + 1)],
                        qt[:], start=True, stop=True)
                    nc.tensor.matmul(
                        sp[:, 512:768],
                        kvts[gg1 // 4][0:64, 128 * (gg1 % 4):128 * (gg1 % 4 + 1)],
                        qt[:, 256:512], start=True, stop=True)
                    wtd = wtd_pool.tile([P, 768], BF16, tag="wtd")
                    if True:
                        # ScalarE exp + DVE mask (keeps DVE lighter late)
                        nc.scalar.activation(
                            wtd[:], sp[:, 0:768],
                            mybir.ActivationFunctionType.Exp,
                            scale=1.0 / math.sqrt(H))
                        nc.vector.tensor_mul(wtd[:], wtd[:], mskb_sb[:])
                    else:
                        se = si_pool.tile([P, 768], BF16, tag="si")
                        nc.vector.tensor_scalar(
                            se[:].bitcast(I16), sp[:, 0:768], SA16, SB16,
                            mybir.AluOpType.mult, mybir.AluOpType.add)
                        nc.vector.tensor_mul(wtd[:], se[:], mskb_sb[:])
                    return wtd

                def outs_diag(ops, j, wtd, last, st=st):
                    gg0, gg1 = 2 * j, 2 * j + 1
                    def fn():
                        for d in range(4):
                            out_mm(st, ops, j, gg0, d,
                                   wtd[:, 128 * d:128 * (d + 1)], stop=False)
                        for d in range(2, 4):
                            out_mm(st, ops, j, gg1, d,
                                   wtd[:, 512 + 128 * (d - 2):512 + 128 * (d - 1)],
                                   stop=(last and d == 3))
                        if last:
                            finish_j(ops, j)
                    return fn

                if j % 2 == 0:
                    emit_kv_block(j // 2)
                ps = pps.tile([64, 512], F32, tag="pps")
                for ci in range(CC):
                    nc.tensor.matmul(
                        ps[:], wq_sb[:, 64 * ci:64 * (ci + 1)],
                        xt[:, ci, 512 * j:512 * (j + 1)],
                        start=(ci == 0), stop=(ci == CC - 1))
                qt = big.tile([64, 512], BF16, tag=f"qT{j}")
                nc.vector.tensor_copy(qt[:], ps[:])

                ops = ops_pool.tile([P, 4 * 65], F32, tag="ops")

                if j == NJ - 1:
                    # last tile: diagonal group first to shorten the tail;
                    # middle groups exp on DVE (ScalarE is tail-bound)
                    wtd = emit_diag(qt, j)
                    pending.append(outs_diag(ops, j, wtd, last=False))
                    flush_pending(2)
                    for g in range(j):
                        wt = emit_scores_nondiag(qt, g, j,
                                                 dve=(g % 2 == 1))
                        pending.append(outs_nondiag(ops, j, g, wt,
                                                    last=(g == j - 1)))
                        flush_pending(2)
                else:
                    for g in range(j):
                        wt = emit_scores_nondiag(qt, g, j,
                                                 dve=(g % 2 == 1))
                        pending.append(outs_nondiag(ops, j, g, wt, last=False))
                        flush_pending(2)
                    wtd = emit_diag(qt, j)
                    pending.append(outs_diag(ops, j, wtd, last=True))
                    flush_pending(2)
                if j <= 1:
                    warm_fill(8)

            flush_pending(0)

    nc.compile()
    return nc


def get_nc():
    if "nc" not in _NC_CACHE:
        _NC_CACHE["nc"] = _build_nc()
    return _NC_CACHE["nc"]


def make_in_maps(x, Wq, Wk, Wv):
    bf = ml_dtypes.bfloat16
    wq_in = np.zeros((P, CC * 64), bf)
    wkv_in = np.zeros((P, CC * 128), bf)
    for ci in range(CC):
        wq_in[:, 64 * ci:64 * (ci + 1)] = Wq[P * ci:P * (ci + 1), :].astype(bf)
        wkv_in[:, 128 * ci:128 * ci + 64] = Wk[P * ci:P * (ci + 1), :].astype(bf)
        wkv_in[:, 128 * ci + 64:128 * (ci + 1)] = Wv[P * ci:P * (ci + 1), :].astype(bf)
    si = np.arange(P)[:, None]
    ti = np.arange(256)[None, :]
    in_maps = []
    rows = np.arange(T)
    for c in range(NCORES):
        b, p = c // 2, c % 2
        xb = np.asarray(x[b], dtype=np.float32)
        # x6[p, ci, t] = x[b][t, 128ci + p]
        x6 = np.ascontiguousarray(
            xb.T.reshape(CC, P, T).transpose(1, 0, 2)).astype(bf)
        own = rows[(rows // P) % 2 == p]
        xo = xb[own].T.reshape(CC, P, OWN).transpose(1, 0, 2)
        xo6 = np.ascontiguousarray(xo).astype(bf)
        # Masks for the diagonal group: cols 0:256 chunk-2j subblocks d0,d1;
        # 256:512 ones (chunk 2j d2,d3); 512:768 chunk-2j+1 subblocks d2,d3.
        m0 = ((si + P * p) <= ti).astype(np.float32)
        m1 = ((si + P * p) <= ti).astype(np.float32)
        ones = np.ones((P, 256), np.float32)
        msk_in = np.ascontiguousarray(np.concatenate([m0, ones, m1], axis=1))
        in_maps.append({"x6": x6, "xo6": xo6, "wq": wq_in,
                        "wkv": wkv_in,
                        "mskb": msk_in.astype(ml_dtypes.bfloat16)})
    return in_maps


def combine(results, B=4):
    out = np.zeros((B, T, H), np.float32)
    for b in range(B):
        o = results[2 * b]["outp"].astype(np.float32) \
            + results[2 * b + 1]["outp"].astype(np.float32)
        o = o.reshape(NJ, P, 4, 65).transpose(0, 2, 1, 3).reshape(T, 65)
        out[b] = o[:, :64] / o[:, 64:65]
    return out


def kernel(x, Wq, Wk, Wv, **run_kwargs):
    nc = get_nc()
    in_maps = make_in_maps(x, Wq, Wk, Wv)
    res = None
    for attempt in range(3):
        try:
            res = bass_utils.run_bass_kernel_spmd(nc, in_maps,
                                                  list(range(NCORES)),
                                                  **run_kwargs)
            break
        except Exception:
            # Transient NRT device errors have been observed on the first
            # run after a fresh compile; retry.
            if attempt == 2:
                raise
    out = combine(res.results, B=x.shape[0])
    if run_kwargs:
        kernel.last_results = res
    return out


# revision 39
# speedup vs baseline: 1.0182x; 1.0182x over previous
"""Single-head causal attention (B=4, T=4096, C=768, H=64) on 8 NeuronCores.

Sharding: 2 cores per batch. Within a batch the 4096 keys are split between
the two cores by interleaved 128-row blocks (core parity p takes global key
blocks {2g+p}).  Every core computes partial attention (un-normalized
numerator + denominator) for ALL 4096 queries of its batch over ITS 2048
keys; the host adds the two partials and normalizes.  This makes the causal
work exactly equal on all 8 cores and the device program identical (all
core-dependence lives in the input data, including the diagonal masks).

Device program (per core), all matmul operands bf16, accumulation fp32:
  qT  [64, 512]  = Wq^T @ x^T per q-tile j     (x^T supplied as [128,6,T])
  kvT [128, 512] = [Wk|Wv]^T @ x_own^T per own 512-key block
  v   [s,64]     = PE-transpose of vT rows, + ones column -> v' [s,65]
  per q-tile j (512 queries), own key chunks gg <= 2j+1 (128 keys each):
     scoresT[s,q] = kT_g^T @ qT_j  (PSUM [128, 512] per chunk)
     non-diagonal groups of 2 chunks: exp(s/8) alternating between
       ScalarE (real exp) and DVE (one-op Schraudolph exp straight to
       bf16: i16 = s*A/2^16 + B/2^16; the int16 bit pattern IS the bf16
       value).  The diagonal group (chunks 2j, 2j+1) uses ScalarE exp +
       a DVE multiply by the causal mask; chunk 2j+1 only needs
       q-subblocks d>=2 (256 cols).
  out flip: ops[128q, 65] += w[128s, 128q-slice]^T @ v'[128s, 65]
     accumulated over chunks per q-subblock d (cols 65d..65d+64; col 64 =
     denominator via the ones column).  One PSUM group per j-tile (start on
     first emitted matmul, stop on last; zero-region = the whole 2KB bank).

Scheduling: the out-matmuls of each score-group are deferred two groups so
the static PE stream never waits inline for an exp; the last j-tile runs
its diagonal group first to shorten the tail; dummy warm-up matmuls run
during the initial DMA wait so the PE reaches full clock before real work;
inputs stream in as a few large, arrival-ordered DMAs.
"""

import sys

for _p in ("/opt/trn_rl_repo",):
    if _p not in sys.path:
        sys.path.insert(0, _p)

import math
import numpy as np
import ml_dtypes

import concourse.bass as bass
import concourse.mybir as mybir
import concourse.tile as tile
from concourse import bacc
from concourse import bass_utils
from concourse.masks import make_identity

BF16 = mybir.dt.bfloat16
F32 = mybir.dt.float32
I32 = mybir.dt.int32
I16 = mybir.dt.int16

P = 128
T = 4096
C = 768
H = 64
CC = C // P        # 6 contraction chunks
OWN = T // 2       # own keys per core
NJ = T // 512      # 8 q-tiles
NCORES = 8

# Schraudolph exp(s/8) straight to bf16: i16 = s*SA16 + SB16 truncated;
# the int16 bit pattern is the bf16 value (top half of the f32 word).
SA16 = float((1 << 23) / (8.0 * math.log(2.0)) / 65536.0)
SB16 = float((127 * (1 << 23) - 366000) / 65536.0 + 0.5)

_NC_CACHE = {}


def _build_nc():
    nc = bacc.Bacc("TRN2", target_bir_lowering=False, debug=False,
                   num_devices=NCORES)

    x6 = nc.dram_tensor("x6", [P, CC, T], BF16, kind="ExternalInput")
    xo6 = nc.dram_tensor("xo6", [P, CC, OWN], BF16, kind="ExternalInput")
    wq = nc.dram_tensor("wq", [P, CC * 64], BF16, kind="ExternalInput")
    wkv = nc.dram_tensor("wkv", [P, CC * 128], BF16, kind="ExternalInput")
    mskb = nc.dram_tensor("mskb", [P, 768], BF16, kind="ExternalInput")
    outp = nc.dram_tensor("outp", [NJ, P, 4 * 65], F32, kind="ExternalOutput")

    with tile.TileContext(nc) as tc:
        with (
            tc.tile_pool(name="const", bufs=1) as cst,
            tc.tile_pool(name="big", bufs=1) as big,
            tc.tile_pool(name="pps", bufs=2, space="PSUM") as pps,
            tc.tile_pool(name="sps", bufs=2, space="PSUM") as sps_pool,
            tc.tile_pool(name="ops", bufs=2, space="PSUM") as ops_pool,
            tc.tile_pool(name="wt", bufs=4) as wt_pool,
            tc.tile_pool(name="wtd", bufs=2) as wtd_pool,
            tc.tile_pool(name="si", bufs=2) as si_pool,
            tc.tile_pool(name="osb", bufs=4) as osb_pool,
        ):
            ident = cst.tile([P, P], BF16)
            make_identity(nc, ident[:])
            wkv_sb = cst.tile([P, CC * 128], BF16)
            nc.sync.dma_start(wkv_sb[:], wkv[:])
            wq_sb = cst.tile([P, CC * 64], BF16)
            mskb_sb = cst.tile([P, 768], BF16)

            xt = big.tile([P, CC, T], BF16, tag="xt")
            xto = big.tile([P, CC, OWN], BF16, tag="xto")

            # Paced input stream (q-tile j: xt cols [512j,512j+512); kv
            # block tb: xto cols [512tb,512tb+512)).  wkv/xto_b0 lead so the
            # kv-projection can start first; wq/msk follow xt_j0.
            nc.sync.dma_start(xto[:, :, 0:512], xo6[:, :, 0:512])
            nc.sync.dma_start(xt[:, :, 0:512], x6[:, :, 0:512])
            nc.sync.dma_start(wq_sb[:], wq[:])
            nc.sync.dma_start(mskb_sb[:], mskb[:])
            for dst, src, a, b in (
                (xt, x6, 512, 1024),
                (xt, x6, 1024, 1536),
                (xto, xo6, 512, 1024),
                (xt, x6, 1536, 2048),
                (xt, x6, 2048, 2560),
                (xto, xo6, 1024, 1536),
                (xt, x6, 2560, 3072),
                (xt, x6, 3072, 3584),
                (xto, xo6, 1536, 2048),
                (xt, x6, 3584, 4096),
            ):
                nc.sync.dma_start(dst[:, :, a:b], src[:, :, a:b])

            # PE warm-up during the initial DMA wait: cheap matmuls on the
            # identity keep the PE continuously busy so it ramps to full
            # clock before real data arrives.  Results are discarded.
            wps = pps.tile([P, 512], F32, tag="pps")
            for i in range(30):
                nc.tensor.matmul(wps[:, 0:P], ident[:], ident[:],
                                 start=True, stop=True)

            kvts = []
            vsbs = []

            def emit_kv_block(tb):
                ps = pps.tile([P, 512], F32, tag="pps")
                for ci in range(CC):
                    nc.tensor.matmul(
                        ps[:], wkv_sb[:, 128 * ci:128 * (ci + 1)],
                        xto[:, ci, 512 * tb:512 * (tb + 1)],
                        start=(ci == 0), stop=(ci == CC - 1))
                kvt = big.tile([P, 512], BF16, tag=f"kvT{tb}")
                nc.vector.tensor_copy(kvt[:], ps[:])
                kvts.append(kvt)
                # v' tiles for the 4 chunks of this block
                vsb = big.tile([P, 4 * 65], BF16, tag=f"v{tb}")
                nc.gpsimd.memset(vsb[:], 1.0)
                vp = pps.tile([P, 512], BF16, tag="pps")
                for i in range(4):
                    nc.tensor.transpose(
                        vp[:, 64 * i:64 * (i + 1)],
                        kvt[64:128, 128 * i:128 * (i + 1)],
                        ident[64:128, 64:128])
                for i in range(4):
                    nc.vector.tensor_copy(vsb[:, 65 * i:65 * i + 64],
                                          vp[:, 64 * i:64 * (i + 1)])
                vsbs.append(vsb)

            def warm_fill(n):
                w = pps.tile([P, 512], F32, tag="pps")
                for _ in range(n):
                    nc.tensor.matmul(w[:, 0:P], ident[:], ident[:],
                                     start=True, stop=True)

            # Deferred out-matmul work items, flushed two score-groups late
            # so the PE stream never waits inline on an exp.
            pending = []

            def flush_pending(limit):
                while len(pending) > limit:
                    pending.pop(0)()

            def out_mm(st, ops, j, gg, d, lhsT, stop):
                nc.tensor.matmul(
                    ops[:, 65 * d:65 * (d + 1)], lhsT,
                    vsbs[gg // 4][:, 65 * (gg % 4):65 * (gg % 4 + 1)],
                    start=not st["started"], stop=stop)
                st["started"] = True

            for j in range(NJ):
                st = {"started": False}

                def finish_j(ops, j):
                    osb = osb_pool.tile([P, 4 * 65], F32, tag="osb")
                    if j >= 6:
                        nc.vector.tensor_copy(osb[:], ops[:])
                    else:
                        nc.scalar.copy(osb[:], ops[:])
                    nc.sync.dma_start(outp[j], osb[:])

                def emit_scores_nondiag(qt, g, j, dve=False):
                    sp = sps_pool.tile([P, 1024], F32, tag="sps")
                    for i in range(2):
                        gg = 2 * g + i
                        nc.tensor.matmul(
                            sp[:, 512 * i:512 * (i + 1)],
                            kvts[gg // 4][0:64, 128 * (gg % 4):128 * (gg % 4 + 1)],
                            qt[:], start=True, stop=True)
                    wt = wt_pool.tile([P, 1024], BF16, tag="wt")
                    if dve:
                        nc.vector.tensor_scalar(
                            wt[:].bitcast(I16), sp[:], SA16, SB16,
                            mybir.AluOpType.mult, mybir.AluOpType.add)
                    else:
                        nc.scalar.activation(
                            wt[:], sp[:], mybir.ActivationFunctionType.Exp,
                            scale=1.0 / math.sqrt(H))
                    return wt

                def outs_nondiag(ops, j, g, wt, last, st=st):
                    def fn():
                        for i in range(2):
                            gg = 2 * g + i
                            for d in range(4):
                                out_mm(st, ops, j, gg, d,
                                       wt[:, 512 * i + 128 * d:
                                          512 * i + 128 * (d + 1)],
                                       stop=(last and i == 1 and d == 3))
                        if last:
                            finish_j(ops, j)
                    return fn

                def emit_diag(qt, j):
                    sp = sps_pool.tile([P, 1024], F32, tag="sps")
                    gg0, gg1 = 2 * j, 2 * j + 1
                    nc.tensor.matmul(
                        sp[:, 0:512],
                        kvts[gg0 // 4][0:64, 128 * (gg0 % 4):128 * (gg0 % 4 + 1)],
                        qt[:], start=True, stop=True)
                    nc.tensor.matmul(
                        sp[:, 512:768],
                        kvts[gg1 // 4][0:64, 128 * (gg1 % 4):128 * (gg1 % 4 + 1)],
                        qt[:, 256:512], start=True, stop=True)
                    wtd = wtd_pool.tile([P, 768], BF16, tag="wtd")
                    if True:
                        # ScalarE exp + DVE mask (keeps DVE lighter late)
                        nc.scalar.activation(
                            wtd[:], sp[:, 0:768],
                            mybir.ActivationFunctionType.Exp,
                            scale=1.0 / math.sqrt(H))
                        nc.vector.tensor_mul(wtd[:], wtd[:], mskb_sb[:])
                    else:
                        se = si_pool.tile([P, 768], BF16, tag="si")
                        nc.vector.tensor_scalar(
                            se[:].bitcast(I16), sp[:, 0:768], SA16, SB16,
                            mybir.AluOpType.mult, mybir.AluOpType.add)
                        nc.vector.tensor_mul(wtd[:], se[:], mskb_sb[:])
                    return wtd

                def outs_diag(ops, j, wtd, last, st=st):
                    gg0, gg1 = 2 * j, 2 * j + 1
                    def fn():
                        for d in range(4):
                            out_mm(st, ops, j, gg0, d,
                                   wtd[:, 128 * d:128 * (d + 1)], stop=False)
                        for d in range(2, 4):
                            out_mm(st, ops, j, gg1, d,
                                   wtd[:, 512 + 128 * (d - 2):512 + 128 * (d - 1)],
                                   stop=(last and d == 3))
                        if last:
                            finish_j(ops, j)
                    return fn

                if j % 2 == 0:
                    emit_kv_block(j // 2)
                ps = pps.tile([64, 512], F32, tag="pps")
                for ci in range(CC):
                    nc.tensor.matmul(
                        ps[:], wq_sb[:, 64 * ci:64 * (ci + 1)],
                        xt[:, ci, 512 * j:512 * (j + 1)],
                        start=(ci == 0), stop=(ci == CC - 1))
                qt = big.tile([64, 512], BF16, tag=f"qT{j}")
                nc.vector.tensor_copy(qt[:], ps[:])

                ops = ops_pool.tile([P, 4 * 65], F32, tag="ops")

                if j == NJ - 1:
                    # last tile: diagonal group first to shorten the tail;
                    # middle groups exp on DVE (ScalarE is tail-bound)
                    wtd = emit_diag(qt, j)
                    pending.append(outs_diag(ops, j, wtd, last=False))
                    flush_pending(2)
                    for g in range(j):
                        wt = emit_scores_nondiag(qt, g, j,
                                                 dve=(g % 2 == 1))
                        pending.append(outs_nondiag(ops, j, g, wt,
                                                    last=(g == j - 1)))
                        flush_pending(2)
                else:
                    for g in range(j):
                        wt = emit_scores_nondiag(qt, g, j,
                                                 dve=(g % 2 == 1))
                        pending.append(outs_nondiag(ops, j, g, wt, last=False))
                        flush_pending(2)
                    wtd = emit_diag(qt, j)
                    pending.append(outs_diag(ops, j, wtd, last=True))
                    flush_pending(2)
                if j <= 1:
                    warm_fill(8)

            flush_pending(0)

    nc.compile()
    return nc


def get_nc():
    if "nc" not in _NC_CACHE:
        _NC_CACHE["nc"] = _build_nc()
    return _NC_CACHE["nc"]


def make_in_maps(x, Wq, Wk, Wv):
    bf = ml_dtypes.bfloat16
    wq_in = np.zeros((P, CC * 64), bf)
    wkv_in = np.zeros((P, CC * 128), bf)
    for ci in range(CC):
        wq_in[:, 64 * ci:64 * (ci + 1)] = Wq[P * ci:P * (ci + 1), :].astype(bf)
        wkv_in[:, 128 * ci:128 * ci + 64] = Wk[P * ci:P * (ci + 1), :].astype(bf)
        wkv_in[:, 128 * ci + 64:128 * (ci + 1)] = Wv[P * ci:P * (ci + 1), :].astype(bf)
    si = np.arange(P)[:, None]
    ti = np.arange(256)[None, :]
    in_maps = []
    rows = np.arange(T)
    for c in range(NCORES):
        b, p = c // 2, c % 2
        xb = np.asarray(x[b], dtype=np.float32)
        # x6[p, ci, t] = x[b][t, 128ci + p]
        x6 = np.ascontiguousarray(
            xb.T.reshape(CC, P, T).transpose(1, 0, 2)).astype(bf)
        own = rows[(rows // P) % 2 == p]
        xo = xb[own].T.reshape(CC, P, OWN).transpose(1, 0, 2)
        xo6 = np.ascontiguousarray(xo).astype(bf)
        # Masks for the diagonal group: cols 0:256 chunk-2j subblocks d0,d1;
        # 256:512 ones (chunk 2j d2,d3); 512:768 chunk-2j+1 subblocks d2,d3.
        m0 = ((si + P * p) <= ti).astype(np.float32)
        m1 = ((si + P * p) <= ti).astype(np.float32)
        ones = np.ones((P, 256), np.float32)
        msk_in = np.ascontiguousarray(np.concatenate([m0, ones, m1], axis=1))
        in_maps.append({"x6": x6, "xo6": xo6, "wq": wq_in,
                        "wkv": wkv_in,
                        "mskb": msk_in.astype(ml_dtypes.bfloat16)})
    return in_maps


def combine(results, B=4):
    out = np.zeros((B, T, H), np.float32)
    for b in range(B):
        o = results[2 * b]["outp"].astype(np.float32) \
            + results[2 * b + 1]["outp"].astype(np.float32)
        o = o.reshape(NJ, P, 4, 65).transpose(0, 2, 1, 3).reshape(T, 65)
        out[b] = o[:, :64] / o[:, 64:65]
    return out


def kernel(x, Wq, Wk, Wv, **run_kwargs):
    nc = get_nc()
    in_maps = make_in_maps(x, Wq, Wk, Wv)
    res = None
    for attempt in range(3):
        try:
            res = bass_utils.run_bass_kernel_spmd(nc, in_maps,
                                                  list(range(NCORES)),
                                                  **run_kwargs)
            break
        except Exception:
            # Transient NRT device errors have been observed on the first
            # run after a fresh compile; retry.
            if attempt == 2:
                raise
    out = combine(res.results, B=x.shape[0])
    if run_kwargs:
        kernel.last_results = res
    return out
